# revision 13
# baseline (speedup 1.0000x reference)
"""Trainium2 Bass kernel for the fused GNN message-passing block.

Reference computation (per batch b):
    h = silu(x @ W1 + b1) @ W2 + b2                       # [K, C]
    out[q, d, c] = sum_k mask[q,k] * ev[q,k,d] * ef[q,k,c] * h[k,c]

Sharding: data-parallel over (b, q-half) -> 8 cores, each core handles
one b (of 4) and 64 of the 128 q values.  The large per-q tensors are
staged bf16 on the host (official gate is rel_err < 2e-2; this lands
~5e-3), halving the dominant HBM stream.

Measured DMA behavior drives the structure: each dma_start costs
~0.65us of serial descriptor-gen on its queue, and transfers progress
roughly in doorbell order with ~1us completion latency each.  So:
  - ALL constants ride in two host-packed blob DMAs on the scalar
    (HWDGE) queue: blobA fp32 [W1 | xT | b1-transposed], blobB bf16
    [W2 | evT | maskT | b2-row0].
  - the 8 ef chunk loads stream alone on the sync (HWDGE) queue.
  - b1 is folded into the Silu activation's per-partition bias, so
    stage 1 of the MLP is 4 matmul instructions (fp32 doubles on PE).
  - stage 2 runs bf16 (h1s, W2, b2 all bf16), PSUM accumulates fp32.
  - w[k, q, 3] = (mask * ev)^T built on the otherwise-idle gpsimd.
  - main loop per 8-q chunk: DVE multiplies ef by h (bf16 2x mode,
    1.13us), one tiny matmul per q (3-col stationary, tile_position
    col-groups) -> PSUM rows 32*s+d, one ACT drain into o_all.
  - outputs leave in 3 waves of 4 DMAs (per q-residue s): waves after
    chunks 2 and 5 on gpsimd overlap the stream; the final wave is
    spread across queues to parallelize completion latency.

The walrus build in this container accepts at most ONE sync wait per
instruction; _split_multiwaits() hoists extra waits onto single-wait
NOPs (sequencer executes waits in queue order, so this is equivalent).
"""

import numpy as np
import ml_dtypes

import concourse.bass as bass
import concourse.mybir as mybir
import concourse.tile as tile
from concourse.bass import ds, ts
from concourse.bass_utils import run_bass_kernel_spmd

B, Q, K, D, C = 4, 128, 128, 3, 256
N_CORES = 8
QSH = Q // 2  # 64 q rows per core
QB = 8  # q values per ef chunk
NG = QSH // QB  # 8 chunks
F32 = mybir.dt.float32
BF16 = mybir.dt.bfloat16

SBLOB = 1794  # W1 512 | xT 256 | b1T 2 | W2 512 | evT 192 | maskT 64 | b2 256 (row 0)

_NC_CACHE = {}


def _split_multiwaits(nc):
    """Legalize for the 1-sync-wait-per-instruction walrus: hoist all but
    the last wait of each instruction onto single-wait NOPs placed just
    before it on the same engine queue."""
    n = 0
    for f in nc.m.functions:
        for bb in f.blocks:
            out = []
            for inst in bb.instructions:
                si = inst.sync_info
                if si is not None and si.on_wait and len(si.on_wait) > 1:
                    waits = list(si.on_wait)
                    for w in waits[:-1]:
                        n += 1
                        nop = mybir.InstNoOp(
                            name=f"{inst.name}-wsplit{n}", ins=[], outs=[]
                        )
                        nop.engine = inst.engine
                        nop.sync_info = mybir.SyncInfo(on_wait=[w], on_update=[])
                        out.append(nop)
                    inst.sync_info = mybir.SyncInfo(
                        on_wait=[waits[-1]], on_update=list(si.on_update)
                    )
                out.append(inst)
            bb.instructions = out
    return nc


def _build_nc(split=True):
    nc = bass.Bass()

    ef_d = nc.declare_dram_parameter("efT", [K, QSH, C], BF16, isOutput=False)
    blob_d = nc.declare_dram_parameter("blob", [128, SBLOB], BF16, isOutput=False)
    out_d = nc.declare_dram_parameter("out", [4 * D, NG * 2 * C], F32, isOutput=True)

    with tile.TileContext(nc) as tc:
        with (
            tc.tile_pool(name="const", bufs=1) as cpool,
            tc.tile_pool(name="efp", bufs=1) as efpool,
            tc.tile_pool(name="outp", bufs=1) as outpool,
            tc.tile_pool(name="pprep", bufs=1, space="PSUM") as pprep,
            tc.tile_pool(name="pout", bufs=4, space="PSUM") as pout,
        ):
            # ---- sync (HWDGE) queue, strict FIFO: blobs first so they
            # land before the ef stream, then the 8 ef chunks ----
            ones_sb = cpool.tile([1, 128], BF16)
            nc.gpsimd.memset(ones_sb[:], 1.0)
            warmdma = cpool.tile([1, 32], BF16)
            nc.sync.dma_start(warmdma[:], blob_d[0:1, 0:32])
            blob = cpool.tile([128, SBLOB], BF16)
            nc.sync.dma_start(blob[:], blob_d[:, :])
            ef_slots = [
                efpool.tile([K, QB, C], BF16, tag=f"ef{g}", name=f"ef{g}")
                for g in range(NG)
            ]
            for g in range(NG):
                nc.sync.dma_start(ef_slots[g][:], ef_d[:, ts(g, QB), :])

            # ---- dummy Silu on scratch: forces the ACT table load to the
            # head of the scalar queue, off the h critical path ----
            scr_out = cpool.tile([1, 128], F32)
            nc.scalar.activation(
                scr_out[:], ones_sb[:], mybir.ActivationFunctionType.Silu
            )

            # ---- PE warm-up: ~3.4us of bf16 matmuls on scratch while the
            # blob lands flips HAM to 8/8 for the MLP and main loop ----
            w_warm = cpool.tile([128, C], BF16)
            nc.gpsimd.memset(w_warm[:], 0.0)
            warm_ps = pout.tile([128, 2 * C], F32, tag="opsum", name="warm_ps")
            for _ in range(16):
                nc.tensor.matmul(
                    warm_ps[:, :C], w_warm[:, :128], w_warm[:], start=True, stop=True
                )

            # ---- MLP.  Stage 1 bf16: h1T[d, k] = (x @ W1)^T; b1 rides the
            # Silu bias (per-partition, since partitions are d here). ----
            h1T_ps = [
                pprep.tile([128, 128], F32, tag=f"prep{i}", name=f"h1T{i}")
                for i in range(2)
            ]
            for dh in range(2):
                nc.tensor.matmul(
                    h1T_ps[dh][:],
                    blob[:, ds(0 * 256 + dh * 128, 128)],
                    blob[:, ds(512 + 0 * 128, 128)],
                    start=True,
                    stop=False,
                )
                nc.tensor.matmul(
                    h1T_ps[dh][:],
                    blob[:, ds(1 * 256 + dh * 128, 128)],
                    blob[:, ds(512 + 1 * 128, 128)],
                    start=False,
                    stop=True,
                )
            h1sT_sb = cpool.tile([128, 2, 128], BF16)
            for dh in range(2):
                nc.scalar.activation(
                    h1sT_sb[:, dh],
                    h1T_ps[dh][:],
                    mybir.ActivationFunctionType.Silu,
                    bias=blob[:, ds(768 + dh, 1)],
                )
            # Stage 2 bf16: h[k, c] = h1s @ W2 + b2 (rank-1 via ones)
            h_ps = pprep.tile([128, C], F32, tag="hps", name="h_ps")
            nc.tensor.matmul(
                h_ps[:], h1sT_sb[:, 0], blob[:, ds(770, 256)], start=True, stop=False
            )
            nc.tensor.matmul(
                h_ps[:], h1sT_sb[:, 1], blob[:, ds(1026, 256)], start=False, stop=False
            )
            nc.tensor.matmul(
                h_ps[:], ones_sb[:], blob[0:1, ds(1538, 256)], start=False, stop=True
            )
            h_bf = cpool.tile([128, C], BF16)
            nc.scalar.copy(out=h_bf[:], in_=h_ps[:])

            # ---- w[k, q, 3] = (mask * ev)^T, bf16, on the otherwise-idle
            # gpsimd engine so the DVE stays free for the ef*h stream ----
            w_sb = cpool.tile([128, QSH, D], BF16)
            for d in range(D):
                nc.gpsimd.tensor_copy(w_sb[:, :, d], blob[:, ds(1282 + d * 64, 64)])
            nc.gpsimd.tensor_tensor(
                w_sb[:, :, :],
                w_sb[:, :, :],
                blob[:, ds(1474, 64)][:, :, None].to_broadcast([K, QSH, D]),
                mybir.AluOpType.mult,
            )

            # ---- main loop over 8-q chunks; all 64 q outputs staged in
            # o_all, written out in 3 waves ----
            o_all = outpool.tile([128, NG * 2 * C], F32)
            for g in range(NG):
                ef_t = ef_slots[g]
                halves = 2 if g == NG - 1 else 1
                ps = pout.tile([128, 2 * C], F32, tag="opsum", name="ps")
                for hv in range(halves):
                    js = range(hv * QB // halves, (hv + 1) * QB // halves)
                    nc.vector.tensor_tensor(
                        ef_t[:, js.start : js.stop, :],
                        ef_t[:, js.start : js.stop, :],
                        h_bf[:, None, :].to_broadcast([K, len(js), C]),
                        mybir.AluOpType.mult,
                    )
                    for j in js:
                        f, s = j // 4, j % 4
                        q = g * QB + j
                        nc.tensor.matmul(
                            ps[ds(32 * s, D), ds(C * f, C)],
                            w_sb[:, q, :],
                            ef_t[:, j, :],
                            start=True,
                            stop=True,
                            tile_position=(0, 32 * s),
                        )
                    drain_eng = nc.vector if g == NG - 1 else nc.scalar
                    if g == NG - 1:
                        nc.vector.tensor_copy(
                            o_all[
                                :,
                                ds(
                                    g * 2 * C + hv * (2 * C) // halves,
                                    (2 * C) // halves,
                                ),
                            ],
                            ps[:, ds(hv * (2 * C) // halves, (2 * C) // halves)],
                        )
                    else:
                        nc.scalar.copy(
                            out=o_all[
                                :,
                                ds(
                                    g * 2 * C + hv * (2 * C) // halves,
                                    (2 * C) // halves,
                                ),
                            ],
                            in_=ps[:, ds(hv * (2 * C) // halves, (2 * C) // halves)],
                        )
                if g == 5:
                    # overlapped output wave on gpsimd only: the sync ring
                    # must stay clean for the tail of the ef stream
                    hi = (g + 1) * 2 * C
                    for s in range(4):
                        nc.gpsimd.dma_start(
                            out_d[3 * s : 3 * s + 3, :hi],
                            o_all[ds(32 * s, D), :hi],
                        )
            # ---- final wave (chunks 6-7) on the empty scalar/gpsimd rings ----
            lo = 6 * 2 * C
            for s in range(4):
                eng = (nc.scalar, nc.scalar, nc.gpsimd, nc.gpsimd)[s]
                eng.dma_start(out_d[3 * s : 3 * s + 3, lo:], o_all[ds(32 * s, D), lo:])

    return _split_multiwaits(nc) if split else nc


def _get_nc():
    if "nc" not in _NC_CACHE:
        _NC_CACHE["nc"] = _build_nc()
    return _NC_CACHE["nc"]


def _in_maps(inputs):
    x = np.asarray(inputs["x"], dtype=np.float32)
    ev = np.asarray(inputs["ev"], dtype=np.float32)
    ef = np.asarray(inputs["ef"], dtype=np.float32)
    am = np.asarray(inputs["access_mask"], dtype=np.float32)
    W1 = np.asarray(inputs["W1"], dtype=np.float32)
    b1 = np.asarray(inputs["b1"], dtype=np.float32)
    W2 = np.asarray(inputs["W2"], dtype=np.float32)
    b2 = np.asarray(inputs["b2"], dtype=np.float32)
    bf = ml_dtypes.bfloat16

    blob0 = np.zeros((128, SBLOB), dtype=bf)
    for o in range(2):
        blob0[:, o * 256 : (o + 1) * 256] = W1[o * 128 : (o + 1) * 128, :].astype(bf)
        blob0[:, 770 + o * 256 : 770 + (o + 1) * 256] = W2[
            o * 128 : (o + 1) * 128, :
        ].astype(bf)
    blob0[0, 1538:1794] = b2.astype(bf)
    maps = []
    for core in range(N_CORES):
        b, qh = core // 2, core % 2
        sl = slice(qh * QSH, (qh + 1) * QSH)
        bb = blob0.copy()
        xT = x[b].T  # [C, K]
        for o in range(2):
            bb[:, 512 + o * 128 : 512 + (o + 1) * 128] = xT[
                o * 128 : (o + 1) * 128, :
            ].astype(bf)
            bb[:, 768 + o] = b1[o * 128 : (o + 1) * 128].astype(bf)
        evT = ev[b, sl].transpose(1, 2, 0)  # [K, D, QSH]
        for d in range(D):
            bb[:, 1282 + d * 64 : 1282 + (d + 1) * 64] = evT[:, d, :].astype(bf)
        bb[:, 1474:1538] = am[b, sl].T.astype(bf)
        maps.append(
            {
                "efT": np.ascontiguousarray(ef[b, sl].transpose(1, 0, 2).astype(bf)),
                "blob": bb,
            }
        )
    return maps


def _gather(results):
    out = np.empty((B, Q, D, C), dtype=np.float32)
    for core in range(N_CORES):
        b, qh = core // 2, core % 2
        # out DRAM row 3*s+d, col g*512 + f*256 + c  ->  q = g*8 + f*4 + s
        arr = results[core]["out"].reshape(4, D, NG, 2, C)  # [s, d, g, f, c]
        out[b, qh * QSH : (qh + 1) * QSH] = (
            arr.transpose(2, 3, 0, 1, 4).reshape(QSH, D, C)
        )
    return out


def _run(inputs, trace=False, **kwargs):
    nc = _get_nc()
    res = run_bass_kernel_spmd(
        nc, _in_maps(inputs), list(range(N_CORES)), trace=trace, **kwargs
    )
    return _gather(res.results), res


def kernel(**inputs) -> np.ndarray:
    out, _ = _run(inputs, trace=False)
    return out
